# revision 26
# baseline (speedup 1.0000x reference)
"""AFM layer kernel for 8 TRN2 NeuronCores.

Math: the reference's attention softmax is over a size-1 axis, so the
attention weights are exactly 1.0 and the attention MLP (Wa, ba, Wh, bh)
cancels out of the output.  What remains is

    pooled[b, :] = sum_{i<j} e_i * e_j          (elementwise over k=16)
                 = 0.5 * ((sum_f e_f)^2 - sum_f e_f^2)
    out[b]       = sigmoid(pooled @ Wo + bo)

where e_f = emb_tables[f, sparse[b, f], :].  The device kernel is an
embedding gather (indirect DMA, one row per partition per instruction —
the only indirect-DMA shape this toolchain lowers correctly) plus a
small amount of vector math.

The table is augmented host-side to rows [e | e^2] (a data-independent
transform), so a single 128B gather descriptor delivers both the value
and its square — no on-chip squaring, which keeps the scalar engine off
the critical path.

Sharding: data-parallel over batch; each of the 8 cores handles 256 rows
(2 half-tiles of 128 partition rows, batch row = h*128 + p).  Embedding
tables are replicated; Wo/bo ride in the same packed input DMA as the
indices.
"""

import numpy as np

try:
    import concourse  # noqa: F401
except ImportError:  # pragma: no cover
    import sys

    sys.path.insert(0, "/opt/trn_rl_repo")

N_FIELDS = 26
VOCAB = 10000
K = 16
BATCH = 2048
N_CORES = 8
PER_CORE = BATCH // N_CORES  # 256
HALVES = PER_CORE // 128  # 2
N_CHUNK = HALVES * N_FIELDS  # 52 gathered rows per partition
CW = 2 * K  # 32 floats per augmented table row [e | e^2]
PACK_W = N_CHUNK + K + 1  # packed input: idx(52) ++ Wo(16) ++ bo(1)

_NC_CACHE = {}


def _build_nc():
    from concourse import bass, mybir

    f32 = mybir.dt.float32
    i32 = mybir.dt.int32

    nc = bass.Bass()
    pack_d = nc.declare_dram_parameter("pack", [128, PACK_W], f32, isOutput=False)
    emb_d = nc.declare_dram_parameter("emb", [N_FIELDS * VOCAB, CW], f32, isOutput=False)
    # out layout: [p, h] — batch row h*128 + p lives at out[p, h]
    out_d = nc.declare_dram_parameter("out", [128, HALVES], f32, isOutput=True)

    with (
        nc.sbuf_tensor([128, PACK_W], f32) as pack_t,
        nc.sbuf_tensor([128, N_CHUNK * CW], f32) as e_t,
        nc.sbuf_tensor([128, HALVES * K], f32) as s_t,
        nc.sbuf_tensor([128, HALVES * K], f32) as q_t,
        nc.sbuf_tensor([128, HALVES * K], f32) as sw_t,
        nc.sbuf_tensor([128, HALVES * K], f32) as ssw_t,
        nc.sbuf_tensor([128, HALVES * K], f32) as qw_t,
        nc.sbuf_tensor([128, HALVES], f32) as t_acc,
        nc.sbuf_tensor([128, HALVES], f32) as u_acc,
        nc.sbuf_tensor([128, HALVES], f32) as d_t,
        nc.sbuf_tensor([128, HALVES], f32) as y_t,
        nc.sbuf_tensor([128, K], f32) as scr_t,
        nc.semaphore("i_sem") as i_sem,
        nc.semaphore("g_sem") as g_sem,
        nc.semaphore("v_sem") as v_sem,
        nc.semaphore("o_sem") as o_sem,
        nc.Block(no_gpsimd_drain=True) as block,
    ):
        idx_v = pack_t[:, 0:N_CHUNK].bitcast(i32)  # int32 bits in f32 carrier
        wo_v = pack_t[:, N_CHUNK : N_CHUNK + K]
        bo_v = pack_t[:, N_CHUNK + K : N_CHUNK + K + 1]

        # e_t free layout per partition: [h, f, (e|sq), k]
        e_all = e_t[:, :].rearrange(
            "p (h f g k) -> p g h k f", h=HALVES, f=N_FIELDS, g=2, k=K
        )
        e_hkf = e_all[:, 0]  # [128, h, k, f] — values
        sq_hkf = e_all[:, 1]  # [128, h, k, f] — squares
        s_v = s_t[:, :].rearrange("p (h k) -> p h k", h=HALVES, k=K)
        q_v = q_t[:, :].rearrange("p (h k) -> p h k", h=HALVES, k=K)
        sw_v = sw_t[:, :].rearrange("p (h k) -> p h k", h=HALVES, k=K)
        ssw_v = ssw_t[:, :].rearrange("p (h k) -> p h k", h=HALVES, k=K)
        qw_v = qw_t[:, :].rearrange("p (h k) -> p h k", h=HALVES, k=K)
        t_v = t_acc[:, :].rearrange("p (h o) -> p h o", h=HALVES, o=1)
        u_v = u_acc[:, :].rearrange("p (h o) -> p h o", h=HALVES, o=1)

        @block.gpsimd
        def _(g):
            g.dma_start(out=pack_t[:, :], in_=pack_d[:, :]).then_inc(i_sem, 16)
            g.wait_ge(i_sem, 16)
            for j in range(N_CHUNK):
                g.indirect_dma_start(
                    out=e_t[:, j * CW : (j + 1) * CW],
                    out_offset=None,
                    in_=emb_d[:, :],
                    in_offset=bass.IndirectOffsetOnAxis(
                        ap=idx_v[:, j : j + 1], axis=0
                    ),
                ).then_inc(g_sem, 16)

        # NOTE on hazards (empirically established on this toolchain):
        # - an instruction reading an SBUF region written by its IMMEDIATELY
        #   preceding same-engine instruction sees stale data (no HW
        #   interlock; engine_nop does not help — real ops do; >=2 real ops
        #   of spacing is verified safe);
        # - a cross-engine consumer gated only by .then_inc on the producing
        #   instruction can also see stale data, so handoff sem incs ride on
        #   a drain preceded by >=2 unrelated real ops.
        @block.scalar
        def _(s):
            s.wait_ge(v_sem, 1)
            s.activation(
                y_t[:, :],
                d_t[:, :],
                func=mybir.ActivationFunctionType.Sigmoid,
                bias=bo_v,
                scale=0.5,
            )
            # wide spacer ops + drain so y_t's write lands, then ACT itself
            # issues the output store (HWDGE) — its ~600ns issue latency adds
            # further margin before the SBUF read
            s.activation(
                scr_t[:, 0:K],
                wo_v,
                func=mybir.ActivationFunctionType.Sigmoid,
                bias=bo_v,
                scale=1.0,
            )
            s.activation(
                scr_t[:, 0:K],
                wo_v,
                func=mybir.ActivationFunctionType.Sigmoid,
                bias=bo_v,
                scale=1.0,
            )
            s.drain()
            s.dma_start(out=out_d[:, :], in_=y_t[:, :]).then_inc(o_sem, 16)
            s.wait_ge(o_sem, 16)

        @block.vector
        def _(v):
            v.wait_ge(i_sem, 16)  # wo available
            # h0 chain while h1 is still gathering
            v.wait_ge(g_sem, 16 * N_FIELDS)
            v.reduce_sum(s_v[:, 0], e_hkf[:, 0], axis=mybir.AxisListType.X)
            v.reduce_sum(q_v[:, 0], sq_hkf[:, 0], axis=mybir.AxisListType.X)
            v.tensor_mul(out=sw_v[:, 0], in0=s_v[:, 0], in1=wo_v)
            v.tensor_mul(out=qw_v[:, 0], in0=q_v[:, 0], in1=wo_v)
            v.tensor_mul(out=ssw_v[:, 0], in0=s_v[:, 0], in1=sw_v[:, 0])
            v.reduce_sum(u_v[:, 0], qw_v[:, 0], axis=mybir.AxisListType.X)
            v.reduce_sum(t_v[:, 0], ssw_v[:, 0], axis=mybir.AxisListType.X)
            # h1 tail
            v.wait_ge(g_sem, 16 * N_CHUNK)
            v.reduce_sum(s_v[:, 1], e_hkf[:, 1], axis=mybir.AxisListType.X)
            v.reduce_sum(q_v[:, 1], sq_hkf[:, 1], axis=mybir.AxisListType.X)
            v.tensor_mul(out=sw_v[:, 1], in0=s_v[:, 1], in1=wo_v)
            v.tensor_mul(out=qw_v[:, 1], in0=q_v[:, 1], in1=wo_v)
            v.tensor_mul(out=ssw_v[:, 1], in0=s_v[:, 1], in1=sw_v[:, 1])
            v.reduce_sum(u_v[:, 1], qw_v[:, 1], axis=mybir.AxisListType.X)
            v.reduce_sum(t_v[:, 1], ssw_v[:, 1], axis=mybir.AxisListType.X)
            # spacers so t_acc's writes land before the sub reads them
            v.tensor_mul(out=qw_v[:, 0], in0=q_v[:, 0], in1=wo_v)
            v.tensor_mul(out=qw_v[:, 1], in0=q_v[:, 1], in1=wo_v)
            v.tensor_sub(out=d_t[:, :], in0=t_acc[:, :], in1=u_acc[:, :])
            # spacers so d_t's write lands before the drain's sem inc
            v.tensor_mul(out=sw_v[:, 0], in0=s_v[:, 0], in1=wo_v)
            v.tensor_mul(out=sw_v[:, 1], in0=s_v[:, 1], in1=wo_v)
            v.drain().then_inc(v_sem, 1)

    return nc


def _get_nc():
    if "nc" not in _NC_CACHE:
        _NC_CACHE["nc"] = _build_nc()
    return _NC_CACHE["nc"]


def _prep_in_maps(sparse, emb_tables, Wo, bo):
    sparse = np.asarray(sparse)
    emb_flat = np.asarray(emb_tables, dtype=np.float32).reshape(N_FIELDS * VOCAB, K)
    emb_aug = np.empty((N_FIELDS * VOCAB, CW), dtype=np.float32)
    emb_aug[:, 0:K] = emb_flat
    emb_aug[:, K:CW] = emb_flat * emb_flat

    # flat row index into the stacked [26*10000, 32] table
    flat_idx = (
        sparse.astype(np.int32) + (np.arange(N_FIELDS, dtype=np.int32) * VOCAB)[None, :]
    )  # [2048, 26]

    wo_row = np.asarray(Wo, dtype=np.float32).reshape(K)
    bo_val = np.float32(np.asarray(bo).reshape(-1)[0])

    in_maps = []
    for c in range(N_CORES):
        rows = flat_idx[c * PER_CORE : (c + 1) * PER_CORE]  # [256, 26]
        # [h, p, f] -> [p, (h f)]
        idx_c = np.ascontiguousarray(
            rows.reshape(HALVES, 128, N_FIELDS).transpose(1, 0, 2).reshape(128, N_CHUNK)
        )
        pack = np.empty((128, PACK_W), dtype=np.float32)
        pack[:, 0:N_CHUNK] = idx_c.view(np.float32)
        pack[:, N_CHUNK : N_CHUNK + K] = wo_row[None, :]
        pack[:, N_CHUNK + K] = bo_val
        in_maps.append({"pack": pack, "emb": emb_aug})
    return in_maps


def _run(in_maps, trace=False, **kwargs):
    from concourse.bass_utils import run_bass_kernel_spmd

    nc = _get_nc()
    return run_bass_kernel_spmd(
        nc, in_maps, core_ids=list(range(N_CORES)), trace=trace, **kwargs
    )


def _collect_out(res):
    # res out[c] is [128, HALVES]; batch row c*256 + h*128 + p = out[c][p, h]
    return np.concatenate(
        [res.results[c]["out"].T.reshape(PER_CORE, 1) for c in range(N_CORES)], axis=0
    ).astype(np.float32)


def kernel(dense, sparse, emb_tables, Wa, ba, Wh, bh, Wo, bo):
    in_maps = _prep_in_maps(sparse, emb_tables, Wo, bo)
    res = _run(in_maps)
    return _collect_out(res)
